# revision 12
# baseline (speedup 1.0000x reference)
"""BTV loss kernel for Trainium2 (8 NeuronCores, Bass/Tile).

reference: total = sum over 7x7 neighborhood shifts (k,l) != (0,0) of
           sqrt((x - roll(x,(k,l),axis=(2,3)))**2 + 1e-6).sum()
           out = 0.1 * total / x.size

Math used here:
  - circular-shift symmetry: shift (k,l) and (-k,-l) give identical sums,
    so only the 24 half-space shifts {k>0, any l} u {k==0, l>0} are
    computed and the result doubled.
  - sqrt(d^2 + 1e-6) ~= |d|: relative error of the final sum ~3e-6
    (verified numerically in f64), far below tolerance.

Distribution: pure data parallel over the 24 (b,c) images, 3 images per
core.  Each core returns 128 per-partition partial sums; host reduces in
f64 and applies the scale.
"""

import numpy as np

import concourse.bass as bass
import concourse.bacc as bacc_mod
import concourse.mybir as mybir
from concourse.tile import TileContext
from concourse.bass_utils import run_bass_kernel_spmd

B, C, H, W = 8, 3, 1024, 1024
NCORES = 8
IMGS = (B * C) // NCORES        # images per core = 3
HP = H + 3                      # row-padded (3 wrap rows at bottom)
BASE = 4                        # left col pad (4 keeps bf16 4B alignment)
WP = W + BASE + 3 + 1           # 1032: [w-4..w-1][0..1023][0,1,2][dead]
RB = 128                        # rows per block (partition dim)
NBLK = H // RB                  # 8 row blocks per image
# half-space shifts: (k>0, any l) or (k==0, l>0)
SHIFTS = [(k, l) for k in range(0, 4) for l in range(-3, 4) if (k > 0 or l > 0)]
assert len(SHIFTS) == 24

WEIGHT = 0.1
F32 = mybir.dt.float32


ROWS_BLK = RB + 3  # 131 rows stored per block (128 + 3 halo)


def _build_nc():
    nc = bacc_mod.Bacc("TRN2", target_bir_lowering=False)
    # host layout: x[r, q, i, c] = img_padded[i, 128*r + q, c], q in [0,131)
    X = nc.dram_tensor("x", [NBLK, ROWS_BLK, IMGS, WP], F32, kind="ExternalInput")
    OUT = nc.dram_tensor("out", [128, 1], F32, kind="ExternalOutput")

    nsh = len(SHIFTS)
    with TileContext(nc) as tc:
        with (
            tc.tile_pool(name="ak", bufs=2) as ak_pool,
            tc.tile_pool(name="d", bufs=3) as d_pool,
            tc.tile_pool(name="ab", bufs=2) as ab_pool,
            tc.tile_pool(name="acc", bufs=1) as acc_pool,
        ):
            stage = acc_pool.tile([128, NBLK * nsh], F32)
            touch = acc_pool.tile([128, NBLK], F32)
            for r in range(NBLK):
                # one DMA per row block: akt[p, k, i, c] = X[i, 128r+k+p, c]
                akt = ak_pool.tile([128, 4, IMGS, WP], F32, tag="ak")
                # akt[p,k,i,c] = X[r, p+k, i, c]; rows overlap so the source
                # is partition-strided with a contiguous 4*IMGS*WP free read.
                row = IMGS * WP
                src = bass.AP(
                    X,
                    r * ROWS_BLK * row,
                    [[row, 128], [1, 4 * row]],
                )
                nc.sync.dma_start(out=akt[:], in_=src)
                # cheap DVE read of the fresh tile: absorbs the DMA sem wait
                # so the real TTs stay within the ISA wait-slot limit.
                nc.vector.tensor_copy(
                    out=touch[:, r : r + 1], in_=akt[:, 0, 0, 0:1]
                )
                for si, (k, l) in enumerate(SHIFTS):
                    d = d_pool.tile([128, IMGS, W], F32, tag="d")
                    nc.vector.tensor_tensor(
                        out=d[:],
                        in0=akt[:, 0, :, BASE : BASE + W],
                        in1=akt[:, k, :, BASE + l : BASE + l + W],
                        op=mybir.AluOpType.subtract,
                    )
                    a = ab_pool.tile([128, IMGS, W], F32, tag="ab")
                    col = r * nsh + si
                    nc.scalar.activation(
                        out=a[:],
                        in_=d[:],
                        func=mybir.ActivationFunctionType.Abs,
                        accum_out=stage[:, col : col + 1],
                    )
            part = acc_pool.tile([128, 1], F32)
            nc.vector.tensor_reduce(
                out=part[:],
                in_=stage[:],
                axis=mybir.AxisListType.X,
                op=mybir.AluOpType.add,
            )
            nc.sync.dma_start(out=OUT[:], in_=part[:])
    return nc


_NC = None


def _get_nc():
    global _NC
    if _NC is None:
        _NC = _build_nc()
        if not _NC.is_finalized():
            _NC.finalize()
    return _NC


def _prep_shards(x: np.ndarray) -> list[dict[str, np.ndarray]]:
    """Circularly pad rows/cols and split into 8 per-core shards."""
    imgs = np.ascontiguousarray(x.reshape(B * C, H, W), dtype=np.float32)
    xp = np.empty((B * C, HP, WP), dtype=np.float32)
    xp[:, :H, BASE : BASE + W] = imgs
    xp[:, :H, :BASE] = imgs[:, :, W - BASE :]
    xp[:, :H, BASE + W : BASE + W + 3] = imgs[:, :, :3]
    xp[:, :H, BASE + W + 3 :] = 0.0
    xp[:, H:, :] = xp[:, :3, :]
    shards = xp.reshape(NCORES, IMGS, HP, WP)
    out = []
    for i in range(NCORES):
        t = shards[i].transpose(1, 0, 2)  # (HP, IMGS, WP)
        blk = np.empty((NBLK, ROWS_BLK, IMGS, WP), dtype=np.float32)
        for r in range(NBLK):
            blk[r] = t[r * RB : r * RB + ROWS_BLK]
        out.append({"x": blk})
    return out


def _run(x: np.ndarray, trace: bool = False):
    nc = _get_nc()
    in_maps = _prep_shards(x)
    res = run_bass_kernel_spmd(
        nc, in_maps, core_ids=list(range(NCORES)), trace=trace
    )
    total = 0.0
    for r in res.results:
        total += r["out"].astype(np.float64).sum()
    val = WEIGHT * 2.0 * total / float(B * C * H * W)
    return np.float32(val), res


def kernel(x: np.ndarray) -> np.ndarray:
    x = np.asarray(x, dtype=np.float32)
    val, _ = _run(x, trace=False)
    return val


# revision 16
# speedup vs baseline: 1.3627x; 1.3627x over previous
"""BTV loss kernel for Trainium2 (8 NeuronCores, Bass/Tile).

reference: total = sum over 7x7 neighborhood shifts (k,l) != (0,0) of
           sqrt((x - roll(x,(k,l),axis=(2,3)))**2 + 1e-6).sum()
           out = 0.1 * total / x.size

Math used here:
  - circular-shift symmetry: shift (k,l) and (-k,-l) give identical sums,
    so only the 24 half-space shifts {k>0, any l} u {k==0, l>0} are
    computed and the result doubled.
  - sqrt(d^2 + 1e-6) ~= |d|: relative error of the final sum ~3e-6
    (verified numerically in f64), far below tolerance.
  - bf16 differences: |d| in bf16 adds ~1e-5 relative error (verified).

Pipeline per 128-row block (per core: 3 images x 8 blocks):
  - one DMA loads rows [128r, 128r+131) of all 3 images in bf16, twice
    (even + odd column phase) so every shifted view is 4B-aligned and
    DVE tensor ops run in 2x/4x packed modes.
  - DVE tensor_tensor subtract (bf16, 2x) per shift
  - |d| + free-dim reduce: split between ACT (activation Abs with
    accum_out, 1x but otherwise idle) and DVE (tensor_scalar abs_max 0
    with accum_out, 4x) to balance engine busy time.
  - per-partition partials accumulate in a (128, 192) f32 stage,
    reduced once at the end; host sums 8x128 values in f64.

Distribution: pure data parallel over the 24 (b,c) images, 3 per core.
"""

import numpy as np

import concourse.bass as bass
import concourse.bacc as bacc_mod
import concourse.mybir as mybir
from concourse.tile import TileContext
from concourse.bass_utils import run_bass_kernel_spmd

B, C, H, W = 8, 3, 1024, 1024
NCORES = 8
IMGS = (B * C) // NCORES        # images per core = 3
BASE = 4                        # left col pad (even => 4B-aligned in bf16)
WP = W + BASE + 3 + 1           # 1032: [w-4..w-1][0..1023][0,1,2][pad]
RB = 128                        # rows per block (partition dim)
NBLK = H // RB                  # 8 row blocks per image
ROWS_BLK = RB + 3               # 131 rows stored per block (128 + 3 halo)
# half-space shifts: (k>0, any l) or (k==0, l>0)
SHIFTS = [(k, l) for k in range(0, 4) for l in range(-3, 4) if (k > 0 or l > 0)]
assert len(SHIFTS) == 24
# which shifts get their abs+reduce on DVE (tensor_reduce with
# apply_absolute_value, 1x, ~3262ns) instead of ACT (activation Abs with
# accum_out, 1x, ~3120ns).  DVE also does every subtract (bf16 2x,
# ~1669ns); n=5 balances the two engines at ~57-59us per block.
DVE_ABS = {2, 7, 12, 16, 21}

WEIGHT = 0.1
F32 = mybir.dt.float32
BF16 = mybir.dt.bfloat16


def _build_nc():
    nc = bacc_mod.Bacc("TRN2", target_bir_lowering=False)
    # host layout: x[r, q, j, i, c] = pad_j[i, 128*r + q, c]; j=0 even
    # phase, j=1 odd phase (odd[c] = even[c+1]).
    X = nc.dram_tensor(
        "x", [NBLK, ROWS_BLK, 2, IMGS, WP], BF16, kind="ExternalInput"
    )
    OUT = nc.dram_tensor("out", [128, 1], F32, kind="ExternalOutput")

    nsh = len(SHIFTS)
    row = 2 * IMGS * WP  # elements per stored row q
    with TileContext(nc) as tc:
        with (
            tc.tile_pool(name="ak", bufs=2) as ak_pool,
            tc.tile_pool(name="d", bufs=4) as d_pool,
            tc.tile_pool(name="ab", bufs=4) as ab_pool,
            tc.tile_pool(name="acc", bufs=1) as acc_pool,
        ):
            stage = acc_pool.tile([128, NBLK * nsh], F32)
            touch = acc_pool.tile([128, NBLK], BF16)
            for r in range(NBLK):
                # akt[p,k,j,i,c] = X[r, p+k, j, i, c]; one DMA, source is
                # partition-strided with a contiguous 4*row free read.
                akt = ak_pool.tile([128, 4, 2, IMGS, WP], BF16, tag="ak")
                src = bass.AP(
                    X,
                    r * ROWS_BLK * row,
                    [[row, 128], [1, 4 * row]],
                )
                nc.sync.dma_start(out=akt[:], in_=src)
                # cheap DVE read of the fresh tile: absorbs the DMA sem wait
                nc.vector.tensor_copy(
                    out=touch[:, r : r + 1], in_=akt[:, 0, 0, 0, 0:1]
                )
                for si, (k, l) in enumerate(SHIFTS):
                    d = d_pool.tile([128, IMGS, W], BF16, tag="d")
                    if l % 2 == 0:
                        shifted = akt[:, k, 0, :, BASE + l : BASE + l + W]
                    else:
                        shifted = akt[:, k, 1, :, BASE + l - 1 : BASE + l - 1 + W]
                    nc.vector.tensor_tensor(
                        out=d[:],
                        in0=akt[:, 0, 0, :, BASE : BASE + W],
                        in1=shifted,
                        op=mybir.AluOpType.subtract,
                    )
                    col = r * nsh + si
                    if si in DVE_ABS:
                        nc.vector.tensor_reduce(
                            out=stage[:, col : col + 1],
                            in_=d[:],
                            axis=mybir.AxisListType.XY,
                            op=mybir.AluOpType.add,
                            apply_absolute_value=True,
                        )
                    else:
                        a = ab_pool.tile([128, IMGS, W], BF16, tag="abs")
                        nc.scalar.activation(
                            out=a[:],
                            in_=d[:],
                            func=mybir.ActivationFunctionType.Abs,
                            accum_out=stage[:, col : col + 1],
                        )
            part = acc_pool.tile([128, 1], F32)
            nc.vector.tensor_reduce(
                out=part[:],
                in_=stage[:],
                axis=mybir.AxisListType.X,
                op=mybir.AluOpType.add,
            )
            nc.sync.dma_start(out=OUT[:], in_=part[:])
    return nc


_NC = None


def _get_nc():
    global _NC
    if _NC is None:
        _NC = _build_nc()
        if not _NC.is_finalized():
            _NC.finalize()
    return _NC


def _prep_shards(x: np.ndarray) -> list[dict[str, np.ndarray]]:
    """bf16-cast, circular pad, build even/odd column phases, and blockify
    into the (NBLK, 131, 2, IMGS, WP) per-core device layout."""
    imgs = np.ascontiguousarray(x.reshape(B * C, H, W), dtype=np.float32)

    def to_bf16(a32):
        b = a32.view(np.uint32)
        return ((b + 0x7FFF + ((b >> 16) & 1)) >> 16).astype(np.uint16)

    imgs_b = to_bf16(imgs)  # (24, H, W) uint16 view of bf16
    HPAD = H + 3
    even = np.zeros((B * C, HPAD, WP), dtype=np.uint16)
    even[:, :H, BASE : BASE + W] = imgs_b
    even[:, :H, :BASE] = imgs_b[:, :, W - BASE :]
    even[:, :H, BASE + W : BASE + W + 3] = imgs_b[:, :, :3]
    even[:, H:, :] = even[:, :3, :]
    odd = np.zeros_like(even)
    odd[:, :, :-1] = even[:, :, 1:]

    shards_e = even.reshape(NCORES, IMGS, HPAD, WP)
    shards_o = odd.reshape(NCORES, IMGS, HPAD, WP)
    out = []
    for i in range(NCORES):
        # (HPAD, 2, IMGS, WP)
        t = np.stack([shards_e[i], shards_o[i]], axis=1).transpose(2, 1, 0, 3)
        blk = np.empty((NBLK, ROWS_BLK, 2, IMGS, WP), dtype=np.uint16)
        for r in range(NBLK):
            blk[r] = t[r * RB : r * RB + ROWS_BLK]
        out.append({"x": blk.view(np.dtype("bfloat16") if False else np.uint16)})
    return out


def _run(x: np.ndarray, trace: bool = False):
    import ml_dtypes

    nc = _get_nc()
    in_maps = _prep_shards(x)
    in_maps = [{"x": m["x"].view(ml_dtypes.bfloat16)} for m in in_maps]
    res = run_bass_kernel_spmd(
        nc, in_maps, core_ids=list(range(NCORES)), trace=trace
    )
    total = 0.0
    for r in res.results:
        total += r["out"].astype(np.float64).sum()
    val = WEIGHT * 2.0 * total / float(B * C * H * W)
    return np.float32(val), res


def kernel(x: np.ndarray) -> np.ndarray:
    x = np.asarray(x, dtype=np.float32)
    val, _ = _run(x, trace=False)
    return val


# revision 21
# speedup vs baseline: 1.4684x; 1.0775x over previous
"""BTV loss kernel for Trainium2 (8 NeuronCores, Bass/Tile).

reference: total = sum over 7x7 neighborhood shifts (k,l) != (0,0) of
           sqrt((x - roll(x,(k,l),axis=(2,3)))**2 + 1e-6).sum()
           out = 0.1 * total / x.size

Math used here:
  - circular-shift symmetry: shift (k,l) and (-k,-l) give identical sums,
    so only the 24 half-space shifts {k>0, any l} u {k==0, l>0} are
    computed and the result doubled.
  - sqrt(d^2 + 1e-6) ~= |d|: relative error of the final sum ~3e-6
    (verified numerically in f64), far below tolerance.
  - bf16 differences: |d| in bf16 adds ~1e-5 relative error (verified).

Pipeline per 128-row block (per core: 3 images x 8 blocks):
  - one DMA loads rows [128r, 128r+131) of all 3 images in bf16, twice
    (even + odd column phase) so every shifted view is 4B-aligned and
    DVE tensor ops run in 2x/4x packed modes.
  - DVE tensor_tensor subtract (bf16, 2x) per shift
  - |d| + free-dim reduce: split between ACT (activation Abs with
    accum_out, 1x but otherwise idle) and DVE (tensor_scalar abs_max 0
    with accum_out, 4x) to balance engine busy time.
  - per-partition partials accumulate in a (128, 192) f32 stage,
    reduced once at the end; host sums 8x128 values in f64.

Distribution: pure data parallel over the 24 (b,c) images, 3 per core.
"""

import dataclasses
import re
from operator import add as _py_add

import numpy as np

import concourse.bass as bass
import concourse.bacc as bacc_mod
import concourse.mybir as mybir
from concourse import dve_ops as _dvo
from concourse.dve_spec import AluOp as _DveAluOp
from concourse.dve_spec import Bin, Spec, Src0, Src1
from concourse.tile import TileContext
from concourse.bass_utils import run_bass_kernel_spmd

B, C, H, W = 8, 3, 1024, 1024
NCORES = 8
IMGS = (B * C) // NCORES        # images per core = 3
BASE = 4                        # left col pad (even => 4B-aligned in bf16)
WP = W + BASE + 3 + 1           # 1032: [w-4..w-1][0..1023][0,1,2][pad]
RB = 128                        # rows per block (partition dim)
NBLK = H // RB                  # 8 row blocks per image
ROWS_BLK = RB + 3               # 131 rows stored per block (128 + 3 halo)
# half-space shifts: (k>0, any l) or (k==0, l>0)
SHIFTS = [(k, l) for k in range(0, 4) for l in range(-3, 4) if (k > 0 or l > 0)]
assert len(SHIFTS) == 24
# which shifts run fully on DVE via the fused custom op ABS_DIFF_REDUCE
# (|a-b| + free-dim sum in one 1x instruction, ~3357ns) vs the split path
# (DVE bf16 2x subtract ~1669ns + ACT Abs/accum ~3120ns).  m=7 balances
# the engines: DVE = 7*3357 + 17*1669 ~ 52us/blk, ACT = 17*3120 ~ 53us.
FUSED = {1, 4, 8, 11, 15, 18, 22}

WEIGHT = 0.1
F32 = mybir.dt.float32
BF16 = mybir.dt.bfloat16

_ABSDIFF_OP = None


def _get_absdiff_op():
    """Register (once per process) a custom DVE op:
    out = |in0 - in1|, accum_out = sum(out) along the free dim."""
    global _ABSDIFF_OP
    if _ABSDIFF_OP is not None:
        return _ABSDIFF_OP
    name = "ABS_DIFF_REDUCE_ANT"
    for op in _dvo.OPS:
        if op.name == name:
            _ABSDIFF_OP = op
            return op

    def _ref(in0, in1, s0, s1, imm2):
        b = np.abs(in0.astype(np.float32) - in1.astype(np.float32)).astype(
            np.float32
        )
        return b, b.reshape(b.shape[0], -1).sum(axis=-1, keepdims=True)

    spec = Spec(
        body=Bin(_DveAluOp.ABSOLUTE_DIFF, Src0, Src1),
        accum=_py_add,
        reference=_ref,
    )
    op = _dvo.DveOp(name, spec, subdim=False, uops_sha={})
    _dvo._SUB_OPCODE_FOR_NAME[name] = _dvo._CUSTOM_DVE_ROW_BASE + len(_dvo.OPS)
    shas = {}
    for ver in ("v3", "v4"):
        try:
            op.compile(ver)
            shas[ver] = op.uops_sha.get(ver)
        except ValueError as e:
            m = re.search(r"([0-9a-f]{16})", str(e))
            if not m:
                raise
            shas[ver] = m.group(1)
    op = dataclasses.replace(op, uops_sha=shas)
    _dvo.OPS.append(op)
    _dvo.CUSTOM_DVE_SPECS[name] = spec
    _ABSDIFF_OP = op
    return op


def _build_nc():
    absdiff = _get_absdiff_op()
    nc = bacc_mod.Bacc("TRN2", target_bir_lowering=False)
    # host layout: x[r, q, j, i, c] = pad_j[i, 128*r + q, c]; j=0 even
    # phase, j=1 odd phase (odd[c] = even[c+1]).
    X = nc.dram_tensor(
        "x", [NBLK, ROWS_BLK, 2, IMGS, WP], BF16, kind="ExternalInput"
    )
    OUT = nc.dram_tensor("out", [128, 1], F32, kind="ExternalOutput")

    nsh = len(SHIFTS)
    row = 2 * IMGS * WP  # elements per stored row q
    with TileContext(nc) as tc:
        with (
            tc.tile_pool(name="ak", bufs=2) as ak_pool,
            tc.tile_pool(name="d", bufs=4) as d_pool,
            tc.tile_pool(name="ab", bufs=4) as ab_pool,
            tc.tile_pool(name="acc", bufs=1) as acc_pool,
        ):
            stage = acc_pool.tile([128, NBLK * nsh], F32)
            touch = acc_pool.tile([128, NBLK], BF16)
            for r in range(NBLK):
                # akt[p,k,j,i,c] = X[r, p+k, j, i, c]; one DMA, source is
                # partition-strided with a contiguous 4*row free read.
                akt = ak_pool.tile([128, 4, 2, IMGS, WP], BF16, tag="ak")
                src = bass.AP(
                    X,
                    r * ROWS_BLK * row,
                    [[row, 128], [1, 4 * row]],
                )
                nc.sync.dma_start(out=akt[:], in_=src)
                # cheap DVE read of the fresh tile: absorbs the DMA sem wait
                nc.vector.tensor_copy(
                    out=touch[:, r : r + 1], in_=akt[:, 0, 0, 0, 0:1]
                )
                for si, (k, l) in enumerate(SHIFTS):
                    if l % 2 == 0:
                        shifted = akt[:, k, 0, :, BASE + l : BASE + l + W]
                    else:
                        shifted = akt[:, k, 1, :, BASE + l - 1 : BASE + l - 1 + W]
                    base = akt[:, 0, 0, :, BASE : BASE + W]
                    col = r * nsh + si
                    if si in FUSED:
                        o = ab_pool.tile([128, IMGS, W], BF16, tag="fo")
                        nc.vector._custom_dve(
                            absdiff,
                            out=o[:],
                            in0=base,
                            in1=shifted,
                            accum_out=stage[:, col : col + 1],
                        )
                    else:
                        d = d_pool.tile([128, IMGS, W], BF16, tag="d")
                        nc.vector.tensor_tensor(
                            out=d[:],
                            in0=base,
                            in1=shifted,
                            op=mybir.AluOpType.subtract,
                        )
                        a = ab_pool.tile([128, IMGS, W], BF16, tag="abs")
                        nc.scalar.activation(
                            out=a[:],
                            in_=d[:],
                            func=mybir.ActivationFunctionType.Abs,
                            accum_out=stage[:, col : col + 1],
                        )
            part = acc_pool.tile([128, 1], F32)
            nc.vector.tensor_reduce(
                out=part[:],
                in_=stage[:],
                axis=mybir.AxisListType.X,
                op=mybir.AluOpType.add,
            )
            nc.sync.dma_start(out=OUT[:], in_=part[:])
    return nc


_NC = None


def _get_nc():
    global _NC
    if _NC is None:
        _NC = _build_nc()
        if not _NC.is_finalized():
            _NC.finalize()
    return _NC


def _prep_shards(x: np.ndarray) -> list[dict[str, np.ndarray]]:
    """bf16-cast, circular pad, build even/odd column phases, and blockify
    into the (NBLK, 131, 2, IMGS, WP) per-core device layout."""
    imgs = np.ascontiguousarray(x.reshape(B * C, H, W), dtype=np.float32)

    def to_bf16(a32):
        b = a32.view(np.uint32)
        return ((b + 0x7FFF + ((b >> 16) & 1)) >> 16).astype(np.uint16)

    imgs_b = to_bf16(imgs)  # (24, H, W) uint16 view of bf16
    HPAD = H + 3
    even = np.zeros((B * C, HPAD, WP), dtype=np.uint16)
    even[:, :H, BASE : BASE + W] = imgs_b
    even[:, :H, :BASE] = imgs_b[:, :, W - BASE :]
    even[:, :H, BASE + W : BASE + W + 3] = imgs_b[:, :, :3]
    even[:, H:, :] = even[:, :3, :]
    odd = np.zeros_like(even)
    odd[:, :, :-1] = even[:, :, 1:]

    shards_e = even.reshape(NCORES, IMGS, HPAD, WP)
    shards_o = odd.reshape(NCORES, IMGS, HPAD, WP)
    out = []
    for i in range(NCORES):
        # (HPAD, 2, IMGS, WP)
        t = np.stack([shards_e[i], shards_o[i]], axis=1).transpose(2, 1, 0, 3)
        blk = np.empty((NBLK, ROWS_BLK, 2, IMGS, WP), dtype=np.uint16)
        for r in range(NBLK):
            blk[r] = t[r * RB : r * RB + ROWS_BLK]
        out.append({"x": blk.view(np.dtype("bfloat16") if False else np.uint16)})
    return out


def _run(x: np.ndarray, trace: bool = False):
    import ml_dtypes

    nc = _get_nc()
    in_maps = _prep_shards(x)
    in_maps = [{"x": m["x"].view(ml_dtypes.bfloat16)} for m in in_maps]
    res = run_bass_kernel_spmd(
        nc, in_maps, core_ids=list(range(NCORES)), trace=trace
    )
    total = 0.0
    for r in res.results:
        total += r["out"].astype(np.float64).sum()
    val = WEIGHT * 2.0 * total / float(B * C * H * W)
    return np.float32(val), res


def kernel(x: np.ndarray) -> np.ndarray:
    x = np.asarray(x, dtype=np.float32)
    val, _ = _run(x, trace=False)
    return val


# revision 25
# speedup vs baseline: 1.5038x; 1.0241x over previous
"""BTV loss kernel for Trainium2 (8 NeuronCores, Bass/Tile).

reference: total = sum over 7x7 neighborhood shifts (k,l) != (0,0) of
           sqrt((x - roll(x,(k,l),axis=(2,3)))**2 + 1e-6).sum()
           out = 0.1 * total / x.size

Math used here:
  - circular-shift symmetry: shift (k,l) and (-k,-l) give identical sums,
    so only the 24 half-space shifts {k>0, any l} u {k==0, l>0} are
    computed and the result doubled.
  - sqrt(d^2 + 1e-6) ~= |d|: relative error of the final sum ~3e-6
    (verified numerically in f64), far below tolerance.
  - bf16 differences: |d| in bf16 adds ~1e-5 relative error (verified).

Pipeline per 128-row block (per core: 3 images x 8 blocks):
  - one DMA loads rows [128r, 128r+131) of all 3 images in bf16, twice
    (even + odd column phase) so every shifted view is 4B-aligned and
    DVE tensor ops run in 2x/4x packed modes.
  - DVE tensor_tensor subtract (bf16, 2x) per shift
  - |d| + free-dim reduce: split between ACT (activation Abs with
    accum_out, 1x but otherwise idle) and DVE (tensor_scalar abs_max 0
    with accum_out, 4x) to balance engine busy time.
  - per-partition partials accumulate in a (128, 192) f32 stage,
    reduced once at the end; host sums 8x128 values in f64.

Distribution: pure data parallel over the 24 (b,c) images, 3 per core.
"""

import dataclasses
import re
from operator import add as _py_add

import numpy as np

import concourse.bass as bass
import concourse.bacc as bacc_mod
import concourse.mybir as mybir
from concourse import dve_ops as _dvo
from concourse.dve_spec import AluOp as _DveAluOp
from concourse.dve_spec import Bin, Spec, Src0, Src1
from concourse.tile import TileContext
from concourse.bass_utils import run_bass_kernel_spmd

B, C, H, W = 8, 3, 1024, 1024
NCORES = 8
IMGS = (B * C) // NCORES        # images per core = 3
BASE = 4                        # left col pad (even => 4B-aligned in bf16)
WP = W + BASE + 3 + 1           # 1032: [w-4..w-1][0..1023][0,1,2][pad]
RB = 128                        # rows per block (partition dim)
NBLK = H // RB                  # 8 row blocks per image
ROWS_BLK = RB + 3               # 131 rows stored per block (128 + 3 halo)
# half-space shifts: (k>0, any l) or (k==0, l>0)
SHIFTS = [(k, l) for k in range(0, 4) for l in range(-3, 4) if (k > 0 or l > 0)]
assert len(SHIFTS) == 24
# which shifts run fully on DVE via the fused custom op ABS_DIFF_REDUCE
# (|a-b| + free-dim sum in one 1x instruction, ~3327ns) vs the split path
# (DVE bf16 2x subtract ~1669ns + ACT Abs/accum, batched 4 shifts per
# activation instruction to amortize the ~420ns fixed overhead).
FUSED = {2, 6, 10, 14, 18, 22}
ACT_GROUP = 4

WEIGHT = 0.1
F32 = mybir.dt.float32
BF16 = mybir.dt.bfloat16

_ABSDIFF_OP = None


def _get_absdiff_op():
    """Register (once per process) a custom DVE op:
    out = |in0 - in1|, accum_out = sum(out) along the free dim."""
    global _ABSDIFF_OP
    if _ABSDIFF_OP is not None:
        return _ABSDIFF_OP
    name = "ABS_DIFF_REDUCE_ANT"
    for op in _dvo.OPS:
        if op.name == name:
            _ABSDIFF_OP = op
            return op

    def _ref(in0, in1, s0, s1, imm2):
        b = np.abs(in0.astype(np.float32) - in1.astype(np.float32)).astype(
            np.float32
        )
        return b, b.reshape(b.shape[0], -1).sum(axis=-1, keepdims=True)

    spec = Spec(
        body=Bin(_DveAluOp.ABSOLUTE_DIFF, Src0, Src1),
        accum=_py_add,
        reference=_ref,
    )
    op = _dvo.DveOp(name, spec, subdim=False, uops_sha={})
    _dvo._SUB_OPCODE_FOR_NAME[name] = _dvo._CUSTOM_DVE_ROW_BASE + len(_dvo.OPS)
    shas = {}
    for ver in ("v3", "v4"):
        try:
            op.compile(ver)
            shas[ver] = op.uops_sha.get(ver)
        except ValueError as e:
            m = re.search(r"([0-9a-f]{16})", str(e))
            if not m:
                raise
            shas[ver] = m.group(1)
    op = dataclasses.replace(op, uops_sha=shas)
    _dvo.OPS.append(op)
    _dvo.CUSTOM_DVE_SPECS[name] = spec
    _ABSDIFF_OP = op
    return op


def _build_nc():
    absdiff = _get_absdiff_op()
    nc = bacc_mod.Bacc("TRN2", target_bir_lowering=False)
    # host layout: x[r, q, j, i, c] = pad_j[i, 128*r + q, c]; j=0 even
    # phase, j=1 odd phase (odd[c] = even[c+1]).
    X = nc.dram_tensor(
        "x", [NBLK, ROWS_BLK, 2, IMGS, WP], BF16, kind="ExternalInput"
    )
    OUT = nc.dram_tensor("out", [128, 1], F32, kind="ExternalOutput")

    nsh = len(SHIFTS)
    row = 2 * IMGS * WP  # elements per stored row q
    with TileContext(nc) as tc:
        with (
            tc.tile_pool(name="ak", bufs=2) as ak_pool,
            tc.tile_pool(name="d", bufs=2) as d_pool,
            tc.tile_pool(name="ab", bufs=3) as ab_pool,
            tc.tile_pool(name="acc", bufs=1) as acc_pool,
        ):
            stage = acc_pool.tile([128, NBLK * nsh], F32)
            touch = acc_pool.tile([128, NBLK], BF16)
            for r in range(NBLK):
                # akt[p,k,j,i,c] = X[r, p+k, j, i, c]; one DMA, source is
                # partition-strided with a contiguous 4*row free read.
                akt = ak_pool.tile([128, 4, 2, IMGS, WP], BF16, tag="ak")
                src = bass.AP(
                    X,
                    r * ROWS_BLK * row,
                    [[row, 128], [1, 4 * row]],
                )
                nc.sync.dma_start(out=akt[:], in_=src)
                # cheap DVE read of the fresh tile: absorbs the DMA sem wait
                nc.vector.tensor_copy(
                    out=touch[:, r : r + 1], in_=akt[:, 0, 0, 0, 0:1]
                )
                base = akt[:, 0, 0, :, BASE : BASE + W]

                def shifted_ap(si):
                    k, l = SHIFTS[si]
                    if l % 2 == 0:
                        return akt[:, k, 0, :, BASE + l : BASE + l + W]
                    return akt[:, k, 1, :, BASE + l - 1 : BASE + l - 1 + W]

                act_sis = [si for si in range(nsh) if si not in FUSED]
                fused_sis = [si for si in range(nsh) if si in FUSED]
                # interleave: one fused op after each ACT group so both
                # engines stay fed.
                groups = [
                    act_sis[i : i + ACT_GROUP]
                    for i in range(0, len(act_sis), ACT_GROUP)
                ]
                col = r * nsh  # running stage column for this block
                fi = 0
                for gi, grp in enumerate(groups):
                    g = len(grp)
                    dgrp = d_pool.tile([128, ACT_GROUP, IMGS, W], BF16, tag="d")
                    for j, si in enumerate(grp):
                        nc.vector.tensor_tensor(
                            out=dgrp[:, j, :, :],
                            in0=base,
                            in1=shifted_ap(si),
                            op=mybir.AluOpType.subtract,
                        )
                    nc.scalar.activation(
                        out=dgrp[:, :g, :, :],
                        in_=dgrp[:, :g, :, :],
                        func=mybir.ActivationFunctionType.Abs,
                        accum_out=stage[:, col : col + 1],
                    )
                    col += 1
                    if fi < len(fused_sis):
                        si = fused_sis[fi]
                        fi += 1
                        o = ab_pool.tile([128, IMGS, W], BF16, tag="fo")
                        nc.vector._custom_dve(
                            absdiff,
                            out=o[:],
                            in0=base,
                            in1=shifted_ap(si),
                            accum_out=stage[:, col : col + 1],
                        )
                        col += 1
                while fi < len(fused_sis):
                    si = fused_sis[fi]
                    fi += 1
                    o = ab_pool.tile([128, IMGS, W], BF16, tag="fo")
                    nc.vector._custom_dve(
                        absdiff,
                        out=o[:],
                        in0=base,
                        in1=shifted_ap(si),
                        accum_out=stage[:, col : col + 1],
                    )
                    col += 1
            part = acc_pool.tile([128, 1], F32)
            nc.vector.tensor_reduce(
                out=part[:],
                in_=stage[:],
                axis=mybir.AxisListType.X,
                op=mybir.AluOpType.add,
            )
            nc.sync.dma_start(out=OUT[:], in_=part[:])
    return nc


_NC = None


def _get_nc():
    global _NC
    if _NC is None:
        _NC = _build_nc()
        if not _NC.is_finalized():
            _NC.finalize()
    return _NC


def _prep_shards(x: np.ndarray) -> list[dict[str, np.ndarray]]:
    """bf16-cast, circular pad, build even/odd column phases, and blockify
    into the (NBLK, 131, 2, IMGS, WP) per-core device layout."""
    imgs = np.ascontiguousarray(x.reshape(B * C, H, W), dtype=np.float32)

    def to_bf16(a32):
        b = a32.view(np.uint32)
        return ((b + 0x7FFF + ((b >> 16) & 1)) >> 16).astype(np.uint16)

    imgs_b = to_bf16(imgs)  # (24, H, W) uint16 view of bf16
    HPAD = H + 3
    even = np.zeros((B * C, HPAD, WP), dtype=np.uint16)
    even[:, :H, BASE : BASE + W] = imgs_b
    even[:, :H, :BASE] = imgs_b[:, :, W - BASE :]
    even[:, :H, BASE + W : BASE + W + 3] = imgs_b[:, :, :3]
    even[:, H:, :] = even[:, :3, :]
    odd = np.zeros_like(even)
    odd[:, :, :-1] = even[:, :, 1:]

    shards_e = even.reshape(NCORES, IMGS, HPAD, WP)
    shards_o = odd.reshape(NCORES, IMGS, HPAD, WP)
    out = []
    for i in range(NCORES):
        # (HPAD, 2, IMGS, WP)
        t = np.stack([shards_e[i], shards_o[i]], axis=1).transpose(2, 1, 0, 3)
        blk = np.empty((NBLK, ROWS_BLK, 2, IMGS, WP), dtype=np.uint16)
        for r in range(NBLK):
            blk[r] = t[r * RB : r * RB + ROWS_BLK]
        out.append({"x": blk.view(np.dtype("bfloat16") if False else np.uint16)})
    return out


def _run(x: np.ndarray, trace: bool = False):
    import ml_dtypes

    nc = _get_nc()
    in_maps = _prep_shards(x)
    in_maps = [{"x": m["x"].view(ml_dtypes.bfloat16)} for m in in_maps]
    res = run_bass_kernel_spmd(
        nc, in_maps, core_ids=list(range(NCORES)), trace=trace
    )
    total = 0.0
    for r in res.results:
        total += r["out"].astype(np.float64).sum()
    val = WEIGHT * 2.0 * total / float(B * C * H * W)
    return np.float32(val), res


def kernel(x: np.ndarray) -> np.ndarray:
    x = np.asarray(x, dtype=np.float32)
    val, _ = _run(x, trace=False)
    return val
